# revision 3
# baseline (speedup 1.0000x reference)
"""DAGNN K-hop propagation (out = sum_k softmax(att)[k] * A^k x) on 8 TRN2 cores.

Sharding: nodes are split into 8 contiguous shards of 6272 rows (49 blocks of
128). Edges are partitioned by destination-owner core, so each core's
segment-sum is local; a per-hop AllGather of the 6272-row bf16 shard rebuilds
the full feature table each core gathers source rows from (the "halo"
exchange of the 1D partitioning hint — for this uniform random graph the halo
is essentially everything, so a full AllGather is the right collective).

Per hop, per core: dma_gather pulls the ~78k source rows (bf16, 256B each) for
the core's edges from DRAM into SBUF in 128-edge tiles laid out per
(dst-block, src-half) segment; a one-hot segment matrix S per tile (built on
the vector engine via is_equal against an iota row) is matmul'd with the
gathered tile on the tensor engine, accumulating each dst-block's segment sum
in PSUM. PSUM results are scaled by softmax(att)[k] into an f32 output
accumulator and cast to bf16 into the AllGather staging buffer.

dma_gather indices are int16, so the node space is split at 32768: "half A"
edges gather from rows [0, 32768), "half B" from [32768, 50176) with rebased
indices. Pad slots point at row 0 with dst value -1, which the one-hot S maps
to a zero column (garbage * 0 = 0), so pads need no zero rows.
"""
import os
import sys

sys.path.insert(0, "/opt/trn_rl_repo")

import numpy as np
import ml_dtypes

import concourse.bacc as bacc
import concourse.mybir as mybir
from concourse import tile
from concourse.bass_utils import run_bass_kernel_spmd

N_NODES, N_EDGES, D, K = 50000, 625000, 128, 10
CORES, NB = 8, 49
NPC = NB * 128              # 6272 nodes per core
PAD_N = CORES * NPC         # 50176
SPLIT = 32768               # int16 index-window split point
WIN_A, WIN_B = SPLIT, PAD_N - SPLIT
CHUNK = 7                   # dst blocks per gather chunk (NB % CHUNK == 0)
BF16 = ml_dtypes.bfloat16

LAST_EXEC_NS = None         # set when BASS_KERNEL_TRACE=1

_nc_cache = {}


def _host_prep(x, att, edge_index):
    src = np.asarray(edge_index[0], dtype=np.int64)
    dst = np.asarray(edge_index[1], dtype=np.int64)
    gblk = dst >> 7                                  # global 128-block, 0..390
    half = (src >= SPLIT).astype(np.int64)
    seg = gblk * 2 + half
    order = np.argsort(seg, kind="stable")
    seg_s = seg[order]
    counts = np.bincount(seg, minlength=2 * (NB * CORES))
    T_A = int(np.ceil(counts[0::2].max() / 128))
    T_B = int(np.ceil(counts[1::2].max() / 128))
    LA, LB = NB * T_A * 128, NB * T_B * 128

    starts = np.zeros(2 * NB * CORES + 1, np.int64)
    np.cumsum(counts, out=starts[1:])
    rank = np.arange(N_EDGES) - starts[seg_s]
    src_s, dst_s = src[order], dst[order]
    core = gblk[order] // NB
    b = gblk[order] % NB
    mA = seg_s % 2 == 0

    idxA = np.zeros((CORES, LA), np.int16)
    dvA = np.full((CORES, LA), -1.0, np.float32)
    idxB = np.zeros((CORES, LB), np.int16)
    dvB = np.full((CORES, LB), -1.0, np.float32)
    sA = b[mA] * (T_A * 128) + rank[mA]
    idxA[core[mA], sA] = src_s[mA].astype(np.int16)
    dvA[core[mA], sA] = (dst_s[mA] & 127).astype(np.float32)
    mB = ~mA
    sB = b[mB] * (T_B * 128) + rank[mB]
    idxB[core[mB], sB] = (src_s[mB] - SPLIT).astype(np.int16)
    dvB[core[mB], sB] = (dst_s[mB] & 127).astype(np.float32)

    h0 = np.zeros((PAD_N, D), BF16)
    h0[:N_NODES] = x.astype(BF16)
    x_pad = np.zeros((PAD_N, D), np.float32)
    x_pad[:N_NODES] = x.astype(np.float32)
    attw = np.ascontiguousarray(att, dtype=np.float32).reshape(1, K + 1)

    in_maps = []
    for c in range(CORES):
        in_maps.append({
            "h0": h0,
            "xsh": np.ascontiguousarray(x_pad[c * NPC:(c + 1) * NPC]),
            "attw": attw,
            # idx i lives at [i % 16, i // 16], replicated to 8 groups of 16
            # partitions for the Q7 cores.
            "idxA": np.ascontiguousarray(np.tile(idxA[c].reshape(-1, 16).T, (8, 1))),
            "idxB": np.ascontiguousarray(np.tile(idxB[c].reshape(-1, 16).T, (8, 1))),
            # dstv column t holds tile t's per-edge dst-within-block values.
            "dvA": np.ascontiguousarray(dvA[c].reshape(-1, 128).T),
            "dvB": np.ascontiguousarray(dvB[c].reshape(-1, 128).T),
        })
    return T_A, T_B, in_maps


def _build(T_A, T_B, n_iters=K):
    dt = mybir.dt
    LA, LB = NB * T_A * 128, NB * T_B * 128
    nc = bacc.Bacc("TRN2", target_bir_lowering=False, debug=False,
                   num_devices=CORES)
    h0 = nc.dram_tensor("h0", [PAD_N, D], dt.bfloat16, kind="ExternalInput")
    xsh = nc.dram_tensor("xsh", [NPC, D], dt.float32, kind="ExternalInput")
    attw = nc.dram_tensor("attw", [1, K + 1], dt.float32, kind="ExternalInput")
    idxA = nc.dram_tensor("idxA", [128, LA // 16], dt.int16, kind="ExternalInput")
    idxB = nc.dram_tensor("idxB", [128, LB // 16], dt.int16, kind="ExternalInput")
    dvA = nc.dram_tensor("dvA", [128, NB * T_A], dt.float32, kind="ExternalInput")
    dvB = nc.dram_tensor("dvB", [128, NB * T_B], dt.float32, kind="ExternalInput")
    outp = nc.dram_tensor("out", [NPC, D], dt.float32, kind="ExternalOutput")
    iota_c = nc.inline_tensor(
        np.ascontiguousarray(
            np.broadcast_to(np.arange(128, dtype=np.float32), (128, 128))
        ),
        name="iota",
    )

    with tile.TileContext(nc) as tc:
        with (
            tc.tile_pool(name="pers", bufs=1) as pers,
            tc.tile_pool(name="gpool", bufs=2) as gpool,
            tc.tile_pool(name="spool", bufs=24) as spool,
            tc.tile_pool(name="pp", bufs=8, space="PSUM") as pp,
            tc.tile_pool(name="tpool", bufs=4) as tpool,
            tc.tile_pool(name="dram", bufs=2, space="DRAM") as dram,
        ):
            idxA_s = pers.tile([128, LA // 16], dt.int16)
            nc.sync.dma_start(idxA_s[:], idxA[:])
            idxB_s = pers.tile([128, LB // 16], dt.int16)
            nc.sync.dma_start(idxB_s[:], idxB[:])
            dvA_s = pers.tile([128, NB * T_A], dt.float32)
            nc.sync.dma_start(dvA_s[:], dvA[:])
            dvB_s = pers.tile([128, NB * T_B], dt.float32)
            nc.sync.dma_start(dvB_s[:], dvB[:])
            iota_s = pers.tile([128, 128], dt.float32)
            nc.sync.dma_start(iota_s[:], iota_c[:])

            # w = softmax(att) on partition 0, broadcast to all partitions.
            att_s = pers.tile([1, K + 1], dt.float32)
            nc.sync.dma_start(att_s[:], attw[:])
            wexp = pers.tile([1, K + 1], dt.float32)
            nc.scalar.activation(wexp[:], att_s[:],
                                 mybir.ActivationFunctionType.Exp)
            wsum = pers.tile([1, 1], dt.float32)
            nc.vector.tensor_reduce(wsum[:], wexp[:], mybir.AxisListType.X,
                                    mybir.AluOpType.add)
            wrec = pers.tile([1, 1], dt.float32)
            nc.vector.reciprocal(wrec[:], wsum[:])
            wnorm = pers.tile([1, K + 1], dt.float32)
            nc.vector.tensor_scalar_mul(wnorm[:], wexp[:], wrec[:])
            wb = pers.tile([128, K + 1], dt.float32)
            nc.gpsimd.partition_broadcast(wb[:], wnorm[:])

            # out_acc[p, b*D:...] accumulates node (b*128+p)'s output row.
            x_s = pers.tile([128, NB * D], dt.float32)
            nc.sync.dma_start(
                x_s[:].rearrange("p (b f) -> p b f", f=D),
                xsh.ap().rearrange("(b p) f -> p b f", p=128),
            )
            out_acc = pers.tile([128, NB * D], dt.float32)
            nc.vector.tensor_scalar_mul(out_acc[:], x_s[:], wb[:, 0:1])
            hstage = pers.tile([128, NB * D], dt.bfloat16)

            src_t = h0.ap()
            for k in range(1, n_iters + 1):
                for g0 in range(0, NB, CHUNK):
                    nbg = min(CHUNK, NB - g0)
                    nA, nBt = nbg * T_A, nbg * T_B
                    gA = gpool.tile([128, nA * D], dt.bfloat16, tag="gA")
                    gB = gpool.tile([128, nBt * D], dt.bfloat16, tag="gB")
                    nc.gpsimd.dma_gather(
                        gA[:].rearrange("p (t f) -> p t f", f=D),
                        src_t[0:WIN_A, :],
                        idxA_s[:, g0 * T_A * 8:(g0 + nbg) * T_A * 8],
                        nA * 128, nA * 128, D, single_packet=False)
                    nc.gpsimd.dma_gather(
                        gB[:].rearrange("p (t f) -> p t f", f=D),
                        src_t[SPLIT:PAD_N, :],
                        idxB_s[:, g0 * T_B * 8:(g0 + nbg) * T_B * 8],
                        nBt * 128, nBt * 128, D, single_packet=False)
                    for j in range(nbg):
                        b = g0 + j
                        ps = pp.tile([128, D], dt.float32, tag="ps")
                        nmm = T_A + T_B
                        mi = 0
                        for t in range(T_A):
                            S = spool.tile([128, 128], dt.bfloat16, tag="S")
                            nc.vector.tensor_scalar(
                                S[:], iota_s[:],
                                dvA_s[:, b * T_A + t:b * T_A + t + 1], None,
                                mybir.AluOpType.is_equal)
                            nc.tensor.matmul(
                                ps[:], S[:],
                                gA[:, (j * T_A + t) * D:(j * T_A + t + 1) * D],
                                start=(mi == 0), stop=(mi == nmm - 1))
                            mi += 1
                        for t in range(T_B):
                            S = spool.tile([128, 128], dt.bfloat16, tag="S")
                            nc.vector.tensor_scalar(
                                S[:], iota_s[:],
                                dvB_s[:, b * T_B + t:b * T_B + t + 1], None,
                                mybir.AluOpType.is_equal)
                            nc.tensor.matmul(
                                ps[:], S[:],
                                gB[:, (j * T_B + t) * D:(j * T_B + t + 1) * D],
                                start=(mi == 0), stop=(mi == nmm - 1))
                            mi += 1
                        nc.vector.tensor_copy(hstage[:, b * D:(b + 1) * D], ps[:])
                        tmp = tpool.tile([128, D], dt.float32, tag="tmp")
                        nc.vector.tensor_scalar_mul(tmp[:], ps[:], wb[:, k:k + 1])
                        nc.vector.tensor_tensor(
                            out_acc[:, b * D:(b + 1) * D],
                            out_acc[:, b * D:(b + 1) * D], tmp[:],
                            mybir.AluOpType.add)
                    del gA, gB
                if k < n_iters:
                    ag_in = dram.tile([NPC, D], dt.bfloat16, tag="agin")
                    hbuf = dram.tile([PAD_N, D], dt.bfloat16, tag="hbuf")
                    nc.sync.dma_start(
                        ag_in[:].rearrange("(b p) f -> p b f", p=128),
                        hstage[:].rearrange("p (b f) -> p b f", f=D))
                    nc.gpsimd.collective_compute(
                        "AllGather", mybir.AluOpType.bypass,
                        replica_groups=[list(range(CORES))],
                        ins=[ag_in.opt()], outs=[hbuf.opt()])
                    src_t = hbuf[:]
            nc.sync.dma_start(
                outp.ap().rearrange("(b p) f -> p b f", p=128),
                out_acc[:].rearrange("p (b f) -> p b f", f=D))
    nc.compile()
    return nc


def _maybe_install_trace_hook():
    import types
    import antenv
    if "antenv.axon_hooks" in sys.modules:
        return
    hooks = types.ModuleType("antenv.axon_hooks")
    hooks._hook = None
    hooks.set_axon_ntff_profile_hook = lambda h: setattr(hooks, "_hook", h)
    hooks.get_axon_ntff_profile_hook = lambda: hooks._hook
    sys.modules["antenv.axon_hooks"] = hooks
    antenv.axon_hooks = hooks
    try:
        from trn_agent_boot.trn_boot import _ntff_profile_via_ctypes
        hooks.set_axon_ntff_profile_hook(
            _ntff_profile_via_ctypes("/opt/axon/libaxon_pjrt.so"))
    except Exception:
        pass


def kernel(x, att, edge_index):
    global LAST_EXEC_NS
    x = np.asarray(x)
    att = np.asarray(att)
    edge_index = np.asarray(edge_index)
    n_iters = int(os.environ.get("DAGNN_K", K))
    T_A, T_B, in_maps = _host_prep(x, att, edge_index)
    key = (T_A, T_B, n_iters)
    if key not in _nc_cache:
        _nc_cache[key] = _build(T_A, T_B, n_iters)
    nc = _nc_cache[key]
    trace = os.environ.get("BASS_KERNEL_TRACE", "0") == "1"
    if trace:
        _maybe_install_trace_hook()
    res = run_bass_kernel_spmd(nc, in_maps, core_ids=list(range(CORES)),
                               trace=trace)
    LAST_EXEC_NS = res.exec_time_ns
    out = np.concatenate([res.results[c]["out"] for c in range(CORES)], axis=0)
    return np.ascontiguousarray(out[:N_NODES]).astype(np.float32)


# revision 5
# speedup vs baseline: 1.1053x; 1.1053x over previous
"""DAGNN K-hop propagation (out = sum_k softmax(att)[k] * A^k x) on 8 TRN2 cores.

Sharding: nodes are split into 8 contiguous shards of 6272 rows (49 blocks of
128). Edges are partitioned by destination-owner core, so each core's
segment-sum is local; a per-hop AllGather of the 6272-row bf16 shard rebuilds
the full feature table each core gathers source rows from (the "halo"
exchange of the 1D partitioning hint — for this uniform random graph the halo
is essentially everything, so a full AllGather is the right collective).

Per hop, per core: dma_gather pulls the ~78k source rows (bf16, 256B each) for
the core's edges from DRAM into SBUF in 128-edge tiles laid out per
(dst-block, src-half) segment; a one-hot segment matrix S per tile (built on
the vector engine via is_equal against an iota row) is matmul'd with the
gathered tile on the tensor engine, accumulating each dst-block's segment sum
in PSUM. PSUM results are scaled by softmax(att)[k] into an f32 output
accumulator and cast to bf16 into the AllGather staging buffer.

dma_gather indices are int16, so the node space is split at 32768: "half A"
edges gather from rows [0, 32768), "half B" from [32768, 50176) with rebased
indices. Pad slots point at row 0 with dst value -1, which the one-hot S maps
to a zero column (garbage * 0 = 0), so pads need no zero rows.
"""
import os
import sys

sys.path.insert(0, "/opt/trn_rl_repo")

import numpy as np
import ml_dtypes

import concourse.bacc as bacc
import concourse.mybir as mybir
from concourse import tile
from concourse.bass_utils import run_bass_kernel_spmd

N_NODES, N_EDGES, D, K = 50000, 625000, 128, 10
CORES, NB = 8, 49
NPC = NB * 128              # 6272 nodes per core
PAD_N = CORES * NPC         # 50176
SPLIT = 32768               # int16 index-window split point
WIN_A, WIN_B = SPLIT, PAD_N - SPLIT
CHUNK = 7                   # dst blocks per gather chunk (NB % CHUNK == 0)
BF16 = ml_dtypes.bfloat16

LAST_EXEC_NS = None         # set when BASS_KERNEL_TRACE=1
LAST_RESULT = None

_nc_cache = {}


def _host_prep(x, att, edge_index):
    src = np.asarray(edge_index[0], dtype=np.int64)
    dst = np.asarray(edge_index[1], dtype=np.int64)
    gblk = dst >> 7                                  # global 128-block, 0..390
    half = (src >= SPLIT).astype(np.int64)
    seg = gblk * 2 + half
    order = np.argsort(seg, kind="stable")
    seg_s = seg[order]
    counts = np.bincount(seg, minlength=2 * (NB * CORES))
    T_A = int(np.ceil(counts[0::2].max() / 128))
    T_B = int(np.ceil(counts[1::2].max() / 128))
    LA, LB = NB * T_A * 128, NB * T_B * 128

    starts = np.zeros(2 * NB * CORES + 1, np.int64)
    np.cumsum(counts, out=starts[1:])
    rank = np.arange(N_EDGES) - starts[seg_s]
    src_s, dst_s = src[order], dst[order]
    core = gblk[order] // NB
    b = gblk[order] % NB
    mA = seg_s % 2 == 0

    idxA = np.zeros((CORES, LA), np.int16)
    dvA = np.full((CORES, LA), -1.0, np.float32)
    idxB = np.zeros((CORES, LB), np.int16)
    dvB = np.full((CORES, LB), -1.0, np.float32)
    sA = b[mA] * (T_A * 128) + rank[mA]
    idxA[core[mA], sA] = src_s[mA].astype(np.int16)
    dvA[core[mA], sA] = (dst_s[mA] & 127).astype(np.float32)
    mB = ~mA
    sB = b[mB] * (T_B * 128) + rank[mB]
    idxB[core[mB], sB] = (src_s[mB] - SPLIT).astype(np.int16)
    dvB[core[mB], sB] = (dst_s[mB] & 127).astype(np.float32)

    h0 = np.zeros((PAD_N, D), BF16)
    h0[:N_NODES] = x.astype(BF16)
    x_pad = np.zeros((PAD_N, D), np.float32)
    x_pad[:N_NODES] = x.astype(np.float32)
    attw = np.ascontiguousarray(att, dtype=np.float32).reshape(1, K + 1)

    in_maps = []
    for c in range(CORES):
        in_maps.append({
            "h0": h0,
            "xsh": np.ascontiguousarray(x_pad[c * NPC:(c + 1) * NPC]),
            "attw": attw,
            # idx i lives at [i % 16, i // 16], replicated to 8 groups of 16
            # partitions for the Q7 cores.
            "idxA": np.ascontiguousarray(np.tile(idxA[c].reshape(-1, 16).T, (8, 1))),
            "idxB": np.ascontiguousarray(np.tile(idxB[c].reshape(-1, 16).T, (8, 1))),
            # dstv column t holds tile t's per-edge dst-within-block values.
            "dvA": np.ascontiguousarray(dvA[c].reshape(-1, 128).T),
            "dvB": np.ascontiguousarray(dvB[c].reshape(-1, 128).T),
        })
    return T_A, T_B, in_maps


def _build(T_A, T_B, n_iters=K):
    dt = mybir.dt
    LA, LB = NB * T_A * 128, NB * T_B * 128
    nc = bacc.Bacc("TRN2", target_bir_lowering=False, debug=False,
                   num_devices=CORES)
    h0 = nc.dram_tensor("h0", [PAD_N, D], dt.bfloat16, kind="ExternalInput")
    xsh = nc.dram_tensor("xsh", [NPC, D], dt.float32, kind="ExternalInput")
    attw = nc.dram_tensor("attw", [1, K + 1], dt.float32, kind="ExternalInput")
    idxA = nc.dram_tensor("idxA", [128, LA // 16], dt.int16, kind="ExternalInput")
    idxB = nc.dram_tensor("idxB", [128, LB // 16], dt.int16, kind="ExternalInput")
    dvA = nc.dram_tensor("dvA", [128, NB * T_A], dt.float32, kind="ExternalInput")
    dvB = nc.dram_tensor("dvB", [128, NB * T_B], dt.float32, kind="ExternalInput")
    outp = nc.dram_tensor("out", [NPC, D], dt.float32, kind="ExternalOutput")
    iota_c = nc.inline_tensor(
        np.ascontiguousarray(
            np.broadcast_to(np.arange(128, dtype=np.float32), (128, 128))
        ),
        name="iota",
    )

    with tile.TileContext(nc) as tc:
        with (
            tc.tile_pool(name="pers", bufs=1) as pers,
            tc.tile_pool(name="gpool", bufs=2) as gpool,
            tc.tile_pool(name="spool", bufs=2) as spool,
            tc.tile_pool(name="pp", bufs=3, space="PSUM") as pp,
            tc.tile_pool(name="tpool", bufs=4) as tpool,
            tc.tile_pool(name="dram", bufs=2, space="DRAM") as dram,
        ):
            idxA_s = pers.tile([128, LA // 16], dt.int16)
            nc.sync.dma_start(idxA_s[:], idxA[:])
            idxB_s = pers.tile([128, LB // 16], dt.int16)
            nc.sync.dma_start(idxB_s[:], idxB[:])
            dvA_s = pers.tile([128, NB * T_A], dt.float32)
            nc.sync.dma_start(dvA_s[:], dvA[:])
            dvB_s = pers.tile([128, NB * T_B], dt.float32)
            nc.sync.dma_start(dvB_s[:], dvB[:])
            iota_s = pers.tile([128, 128], dt.float32)
            nc.sync.dma_start(iota_s[:], iota_c[:])

            # w = softmax(att) on partition 0, broadcast to all partitions.
            att_s = pers.tile([1, K + 1], dt.float32)
            nc.sync.dma_start(att_s[:], attw[:])
            wexp = pers.tile([1, K + 1], dt.float32)
            nc.scalar.activation(wexp[:], att_s[:],
                                 mybir.ActivationFunctionType.Exp)
            wsum = pers.tile([1, 1], dt.float32)
            nc.vector.tensor_reduce(wsum[:], wexp[:], mybir.AxisListType.X,
                                    mybir.AluOpType.add)
            wrec = pers.tile([1, 1], dt.float32)
            nc.vector.reciprocal(wrec[:], wsum[:])
            wnorm = pers.tile([1, K + 1], dt.float32)
            nc.vector.tensor_scalar_mul(wnorm[:], wexp[:], wrec[:])
            wb = pers.tile([128, K + 1], dt.float32)
            nc.gpsimd.partition_broadcast(wb[:], wnorm[:])

            # out_acc[p, b*D:...] accumulates node (b*128+p)'s output row.
            out_acc = pers.tile([128, NB * D], dt.float32)
            hstage = pers.tile([128, NB * D], dt.bfloat16)
            for g0 in range(0, NB, CHUNK):
                nbg = min(CHUNK, NB - g0)
                xc = tpool.tile([128, nbg * D], dt.float32, tag="tmp")
                nc.sync.dma_start(
                    xc[:].rearrange("p (b f) -> p b f", f=D),
                    xsh.ap()[g0 * 128:(g0 + nbg) * 128, :]
                    .rearrange("(b p) f -> p b f", p=128))
                nc.vector.tensor_scalar_mul(
                    out_acc[:, g0 * D:(g0 + nbg) * D], xc[:], wb[:, 0:1])

            src_t = h0.ap()
            for k in range(1, n_iters + 1):
                for g0 in range(0, NB, CHUNK):
                    nbg = min(CHUNK, NB - g0)
                    nA, nBt = nbg * T_A, nbg * T_B
                    gA = gpool.tile([128, nA * D], dt.bfloat16, tag="gA")
                    gB = gpool.tile([128, nBt * D], dt.bfloat16, tag="gB")
                    nc.gpsimd.dma_gather(
                        gA[:].rearrange("p (t f) -> p t f", f=D),
                        src_t[0:WIN_A, :],
                        idxA_s[:, g0 * T_A * 8:(g0 + nbg) * T_A * 8],
                        nA * 128, nA * 128, D, single_packet=False)
                    nc.gpsimd.dma_gather(
                        gB[:].rearrange("p (t f) -> p t f", f=D),
                        src_t[SPLIT:PAD_N, :],
                        idxB_s[:, g0 * T_B * 8:(g0 + nbg) * T_B * 8],
                        nBt * 128, nBt * 128, D, single_packet=False)
                    # one-hot segment matrices for the whole chunk, one DVE op
                    # per src-half (broadcast APs: iota along tiles, dstv along d)
                    SA = spool.tile([128, nA * 128], dt.bfloat16, tag="SA")
                    nc.vector.tensor_tensor(
                        SA[:].rearrange("p (t d) -> p t d", d=128),
                        iota_s[:].rearrange("p (o d) -> p o d", o=1)
                        .broadcast_to([128, nA, 128]),
                        dvA_s[:, g0 * T_A:(g0 + nbg) * T_A]
                        .rearrange("p (t o) -> p t o", o=1)
                        .broadcast_to([128, nA, 128]),
                        mybir.AluOpType.is_equal)
                    SB = spool.tile([128, nBt * 128], dt.bfloat16, tag="SB")
                    nc.vector.tensor_tensor(
                        SB[:].rearrange("p (t d) -> p t d", d=128),
                        iota_s[:].rearrange("p (o d) -> p o d", o=1)
                        .broadcast_to([128, nBt, 128]),
                        dvB_s[:, g0 * T_B:(g0 + nbg) * T_B]
                        .rearrange("p (t o) -> p t o", o=1)
                        .broadcast_to([128, nBt, 128]),
                        mybir.AluOpType.is_equal)
                    ps = pp.tile([128, nbg * D], dt.float32, tag="ps")
                    for j in range(nbg):
                        nmm = T_A + T_B
                        mi = 0
                        for t in range(T_A):
                            i = j * T_A + t
                            nc.tensor.matmul(
                                ps[:, j * D:(j + 1) * D],
                                SA[:, i * 128:(i + 1) * 128],
                                gA[:, i * D:(i + 1) * D],
                                start=(mi == 0), stop=(mi == nmm - 1))
                            mi += 1
                        for t in range(T_B):
                            i = j * T_B + t
                            nc.tensor.matmul(
                                ps[:, j * D:(j + 1) * D],
                                SB[:, i * 128:(i + 1) * 128],
                                gB[:, i * D:(i + 1) * D],
                                start=(mi == 0), stop=(mi == nmm - 1))
                            mi += 1
                    nc.vector.tensor_copy(
                        hstage[:, g0 * D:(g0 + nbg) * D], ps[:])
                    tmp = tpool.tile([128, nbg * D], dt.float32, tag="tmp")
                    nc.vector.tensor_scalar_mul(tmp[:], ps[:], wb[:, k:k + 1])
                    nc.vector.tensor_tensor(
                        out_acc[:, g0 * D:(g0 + nbg) * D],
                        out_acc[:, g0 * D:(g0 + nbg) * D], tmp[:],
                        mybir.AluOpType.add)
                if k < n_iters:
                    ag_in = dram.tile([NPC, D], dt.bfloat16, tag="agin")
                    hbuf = dram.tile([PAD_N, D], dt.bfloat16, tag="hbuf")
                    nc.sync.dma_start(
                        ag_in[:].rearrange("(b p) f -> p b f", p=128),
                        hstage[:].rearrange("p (b f) -> p b f", f=D))
                    nc.gpsimd.collective_compute(
                        "AllGather", mybir.AluOpType.bypass,
                        replica_groups=[list(range(CORES))],
                        ins=[ag_in.opt()], outs=[hbuf.opt()])
                    src_t = hbuf[:]
            nc.sync.dma_start(
                outp.ap().rearrange("(b p) f -> p b f", p=128),
                out_acc[:].rearrange("p (b f) -> p b f", f=D))
    nc.compile()
    return nc


def _maybe_install_trace_hook():
    import types
    import antenv
    if "antenv.axon_hooks" in sys.modules:
        return
    hooks = types.ModuleType("antenv.axon_hooks")
    hooks._hook = None
    hooks.set_axon_ntff_profile_hook = lambda h: setattr(hooks, "_hook", h)
    hooks.get_axon_ntff_profile_hook = lambda: hooks._hook
    sys.modules["antenv.axon_hooks"] = hooks
    antenv.axon_hooks = hooks
    try:
        from trn_agent_boot.trn_boot import _ntff_profile_via_ctypes
        hooks.set_axon_ntff_profile_hook(
            _ntff_profile_via_ctypes("/opt/axon/libaxon_pjrt.so"))
    except Exception:
        pass


def kernel(x, att, edge_index):
    global LAST_EXEC_NS
    x = np.asarray(x)
    att = np.asarray(att)
    edge_index = np.asarray(edge_index)
    n_iters = int(os.environ.get("DAGNN_K", K))
    T_A, T_B, in_maps = _host_prep(x, att, edge_index)
    key = (T_A, T_B, n_iters)
    if key not in _nc_cache:
        _nc_cache[key] = _build(T_A, T_B, n_iters)
    nc = _nc_cache[key]
    trace = os.environ.get("BASS_KERNEL_TRACE", "0") == "1"
    if trace:
        _maybe_install_trace_hook()
    res = run_bass_kernel_spmd(nc, in_maps, core_ids=list(range(CORES)),
                               trace=trace)
    global LAST_RESULT
    LAST_RESULT = res
    LAST_EXEC_NS = res.exec_time_ns
    out = np.concatenate([res.results[c]["out"] for c in range(CORES)], axis=0)
    return np.ascontiguousarray(out[:N_NODES]).astype(np.float32)
